# revision 1
# baseline (speedup 1.0000x reference)
"""Trainium2 Bass kernel for ConservativeGSAAttention.

Sharding: 8 cores = 4 batches x 2 head-groups (8 heads each).
Each core computes qkv-proj + attention + its half of c_proj for one batch;
the host sums the two partial c_proj outputs per batch (the "all-reduce").

Layout strategy (per core):
  - hidden_states passed transposed: hsT [E, T].
  - qkv proj computed in transposed layout: qT/kT [feat, token] (feat on
    partitions) so that scores matmuls need no transposes; v computed in
    [token, feat] layout for the AV matmul.
  - scores computed transposed: scoresT [keys, queries] (keys on partitions),
    block [128k x 512q]; fully-masked blocks are skipped; exp on ScalarE with
    the per-head splat scale/bias fused in; triangular 128x128 mask applied
    multiplicatively after exp on the diagonal blocks only.
  - softmax denominator comes for free from an appended ones-column in v
    (AV matmul row 64 = sum_k exp); normalization multiplies the AV output
    (64 rows) instead of the attention matrix (2048 rows).
"""

import math

import numpy as np

import concourse.bass as bass
import concourse.tile as tile
from concourse import bacc
from concourse import mybir
from concourse.bass_utils import run_bass_kernel_spmd

B, T, E, H, D = 4, 2048, 1024, 16, 64
HG = 8              # heads per core
F = HG * D          # 512 feats per group (for each of q, k, v)
P = 128
KT = E // P         # 8 contraction tiles for projections
TT = T // P         # 16 token tiles
QB = 512            # query block width
NQB = T // QB       # 4
FT = F // P         # 4 feat tiles per q/k/v group
FT_QK = 2 * FT      # 8 (q tiles then k tiles)
EB = 512            # c_proj output block width
NEB = E // EB       # 2

f32 = mybir.dt.float32
f32r = mybir.dt.float32r

AF = mybir.ActivationFunctionType


def _r(ap):
    """fp32r view of an fp32 AP for full-rate PE matmuls."""
    return ap.bitcast(f32r)


def build_program():
    nc = bacc.Bacc("TRN2", target_bir_lowering=False, debug=False)

    # ---- I/O ----
    hsT = nc.dram_tensor("hsT", [E, T], f32, kind="ExternalInput").ap()
    wqkT = nc.dram_tensor("wqkT", [E, 2 * F], f32, kind="ExternalInput").ap()
    wvT = nc.dram_tensor("wvT", [E, F], f32, kind="ExternalInput").ap()
    qk_bias = nc.dram_tensor("qk_bias", [P, FT_QK], f32, kind="ExternalInput").ap()
    v_bias = nc.dram_tensor("v_bias", [1, F], f32, kind="ExternalInput").ap()
    wpT = nc.dram_tensor("wpT", [F, E], f32, kind="ExternalInput").ap()
    bp_half = nc.dram_tensor("bp_half", [1, E], f32, kind="ExternalInput").ap()
    tri = nc.dram_tensor("tri", [P, P], f32, kind="ExternalInput").ap()
    act_s = nc.dram_tensor("act_s", [P, HG], f32, kind="ExternalInput").ap()
    act_b = nc.dram_tensor("act_b", [P, HG], f32, kind="ExternalInput").ap()
    out = nc.dram_tensor("out", [TT, P, E], f32, kind="ExternalOutput").ap()

    # ---- DRAM scratch ----
    qT_dr = nc.dram_tensor("qT_dr", [FT, P, T], f32).ap()
    kT_dr = nc.dram_tensor("kT_dr", [FT, P, T], f32).ap()
    v_dr = nc.dram_tensor("v_dr", [HG, TT, P, D + 1], f32).ap()
    ao_dr = nc.dram_tensor("ao_dr", [FT, P, T], f32).ap()
    rc_dr = nc.dram_tensor("rc_dr", [HG, NQB, 1, QB], f32).ap()

    from contextlib import ExitStack
    with tile.TileContext(nc) as tc, ExitStack() as ctx:
        def pool(name, bufs, space="SBUF"):
            return ctx.enter_context(tc.tile_pool(name=name, bufs=bufs, space=space))

        consts = pool("consts", 1)
        big = pool("big", 2)
        hs_pool = pool("hs", 2)
        stage = pool("stage", 4)
        vstage = pool("vstage", 2)
        vh_pool = pool("vh", 2)
        kh_pool = pool("kh", 2)
        q_pool = pool("q", 3)
        ao_pool = pool("aostage", 2)
        bc_pool = pool("bcast", 2)
        rc_pool = pool("rc", 2)
        aol_pool = pool("aol", 8)
        out_pool = pool("outp", 2)
        raw_pool = pool("raw", 4)
        msk_pool = pool("msk", 4)
        mm_ps = pool("mm_ps", 2, "PSUM")
        sc_ps = pool("sc_ps", 3, "PSUM")
        av_ps = pool("av_ps", 2, "PSUM")

        if True:
            # ---- resident constants ----
            wqk_sb = big.tile([P, KT, 2 * F], f32r, tag="big")
            nc.sync.dma_start(out=wqk_sb, in_=wqkT.bitcast(f32r).rearrange("(kt p) f -> p kt f", p=P))
            wv_sb = consts.tile([P, KT, F], f32r)
            nc.sync.dma_start(out=wv_sb, in_=wvT.bitcast(f32r).rearrange("(kt p) f -> p kt f", p=P))
            wp_sb = consts.tile([P, FT, E], f32r)
            nc.sync.dma_start(out=wp_sb, in_=wpT.bitcast(f32r).rearrange("(ft p) e -> p ft e", p=P))
            qkb_sb = consts.tile([P, FT_QK], f32)
            nc.sync.dma_start(out=qkb_sb, in_=qk_bias)
            vb_sb = consts.tile([P, 1, F], f32)
            nc.sync.dma_start(out=vb_sb, in_=v_bias.partition_broadcast(P))
            bp_sb = consts.tile([P, 1, E], f32)
            nc.sync.dma_start(out=bp_sb, in_=bp_half.partition_broadcast(P))
            tri_sb = consts.tile([P, P], f32)
            nc.sync.dma_start(out=tri_sb, in_=tri)
            acts_sb = consts.tile([P, HG], f32)
            nc.sync.dma_start(out=acts_sb, in_=act_s)
            actb_sb = consts.tile([P, HG], f32)
            nc.sync.dma_start(out=actb_sb, in_=act_b)
            ones_col = consts.tile([P, 1], f32)
            nc.vector.memset(ones_col, 1.0)

            hsT_t = hsT.rearrange("(kt p) t -> p kt t", p=P)

            # ---- Phase A: qkv projection (transposed layouts) ----
            for tb in range(NQB):
                hs_t = hs_pool.tile([P, KT, QB], f32r)
                nc.sync.dma_start(out=hs_t, in_=hsT_t[:, :, tb * QB:(tb + 1) * QB].bitcast(f32r))

                # qT / kT : [feat, token]
                for ft in range(FT_QK):
                    ps = mm_ps.tile([P, QB], f32, tag="mm")
                    for kt in range(KT):
                        nc.tensor.matmul(
                            ps,
                            (wqk_sb[:, kt, ft * P:(ft + 1) * P]),
                            (hs_t[:, kt, :]),
                            start=(kt == 0),
                            stop=(kt == KT - 1),
                        )
                    st = stage.tile([P, QB], f32r)
                    nc.scalar.activation(
                        out=st, in_=ps, func=AF.Identity,
                        bias=qkb_sb[:, ft:ft + 1], scale=1.0,
                    )
                    if ft < FT:
                        nc.sync.dma_start(
                            out=qT_dr[ft, :, tb * QB:(tb + 1) * QB].bitcast(f32r), in_=st)
                    else:
                        nc.sync.dma_start(
                            out=kT_dr[ft - FT, :, tb * QB:(tb + 1) * QB].bitcast(f32r), in_=st)

                # v : [token, feat] with ones column appended per head
                for tsub in range(QB // P):
                    tt = tb * (QB // P) + tsub
                    psv = mm_ps.tile([P, F], f32, tag="mm")
                    for kt in range(KT):
                        nc.tensor.matmul(
                            psv,
                            (hs_t[:, kt, tsub * P:(tsub + 1) * P]),
                            (wv_sb[:, kt, :]),
                            start=(kt == 0),
                            stop=(kt == KT - 1),
                        )
                    vt = vstage.tile([P, HG, D + 1], f32r)
                    nc.vector.tensor_add(
                        vt[:, :, 0:D],
                        psv.rearrange("p (h d) -> p h d", h=HG),
                        vb_sb.rearrange("p o (h d) -> p (o h) d", h=HG),
                    )
                    for hh in range(HG):
                        nc.vector.tensor_copy(vt[:, hh, D:D + 1], ones_col)
                    nc.sync.dma_start(
                        out=v_dr[:, tt, :, :].bitcast(f32r).rearrange("h p d -> p h d"), in_=vt)

            # ---- Phase B: attention per head ----
            for h in range(HG):
                hf = h // 2          # feat tile holding this head
                hr = (h % 2) * D     # row offset inside the feat tile
                vh = vh_pool.tile([P, TT, D + 1], f32r)
                nc.sync.dma_start(
                    out=vh, in_=v_dr[h, :, :, :].bitcast(f32r).rearrange("tt p d -> p tt d"))
                kh = kh_pool.tile([D, T], f32r)
                nc.sync.dma_start(out=kh, in_=kT_dr[hf, hr:hr + D, :].bitcast(f32r))

                for qb in range(NQB):
                    nkt = (qb + 1) * (QB // P)
                    qt = q_pool.tile([D, QB], f32r)
                    nc.sync.dma_start(
                        out=qt, in_=qT_dr[hf, hr:hr + D, qb * QB:(qb + 1) * QB].bitcast(f32r))

                    at = big.tile([P, NQB * (QB // P), QB], f32r, tag="big")
                    for kt in range(nkt):
                        ps = sc_ps.tile([P, QB], f32, tag="sc")
                        nc.tensor.matmul(
                            ps,
                            (kh[:, kt * P:(kt + 1) * P]),
                            (qt),
                            start=True, stop=True,
                        )
                        j = kt - qb * (QB // P)  # >=0 on diagonal tiles
                        if j < 0:
                            nc.scalar.activation(
                                out=at[:, kt, :], in_=ps, func=AF.Exp,
                                bias=actb_sb[:, h:h + 1], scale=acts_sb[:, h:h + 1],
                            )
                        else:
                            # Keep `at` ACT-only-written (the AV matmul can
                            # carry just one wait): ACT copies the diagonal
                            # 128 cols to SBUF, DVE adds the -1e30 mask
                            # there, ACT exps it back into `at`.
                            raw = raw_pool.tile([P, P], f32)
                            nc.scalar.activation(
                                out=raw, in_=ps[:, j * P:(j + 1) * P],
                                func=AF.Copy)
                            msk = msk_pool.tile([P, P], f32)
                            nc.vector.tensor_add(msk, raw, tri_sb)
                            if j > 0:
                                nc.scalar.activation(
                                    out=at[:, kt, 0:j * P], in_=ps[:, 0:j * P],
                                    func=AF.Copy, scale=0.0)
                            nc.scalar.activation(
                                out=at[:, kt, j * P:(j + 1) * P], in_=msk,
                                func=AF.Exp,
                                bias=actb_sb[:, h:h + 1], scale=acts_sb[:, h:h + 1],
                            )
                            if j < 3:
                                nc.scalar.activation(
                                    out=at[:, kt, (j + 1) * P:QB],
                                    in_=ps[:, (j + 1) * P:QB],
                                    func=AF.Exp,
                                    bias=actb_sb[:, h:h + 1],
                                    scale=acts_sb[:, h:h + 1],
                                )

                    avp = av_ps.tile([D + 1, QB], f32, tag="av")
                    for kt in range(nkt):
                        nc.tensor.matmul(
                            avp,
                            (vh[:, kt, :]),
                            (at[:, kt, :]),
                            start=(kt == 0),
                            stop=(kt == nkt - 1),
                        )

                    # normalize rows 0..63 by row 64 (the exp-sum)
                    rc = rc_pool.tile([1, QB], f32)
                    nc.vector.reciprocal(rc, avp[D:D + 1, :])
                    nc.sync.dma_start(out=rc_dr[h, qb], in_=rc)
                    bc = bc_pool.tile([D, QB], f32)
                    nc.sync.dma_start(
                        out=bc, in_=rc_dr[h, qb].partition_broadcast(D).rearrange(
                            "p o q -> p (o q)"))
                    ao = ao_pool.tile([D, QB], f32)
                    nc.vector.tensor_mul(ao, avp[0:D, :], bc)
                    nc.sync.dma_start(
                        out=ao_dr[hf, hr:hr + D, qb * QB:(qb + 1) * QB], in_=ao)

            # ---- Phase C: c_proj (partial, + bproj/2) ----
            for tt in range(TT):
                ot = out_pool.tile([P, E], f32)
                for eb in range(NEB):
                    ps = mm_ps.tile([P, EB], f32, tag="mm")
                    for ft in range(FT):
                        lt = aol_pool.tile([P, P], f32r)
                        nc.sync.dma_start(
                            out=lt, in_=ao_dr[ft, :, tt * P:(tt + 1) * P].bitcast(f32r))
                        nc.tensor.matmul(
                            ps,
                            (lt),
                            (wp_sb[:, ft, eb * EB:(eb + 1) * EB]),
                            start=(ft == 0),
                            stop=(ft == FT - 1),
                        )
                    nc.vector.tensor_add(
                        ot[:, eb * EB:(eb + 1) * EB], ps,
                        bp_sb[:, 0, eb * EB:(eb + 1) * EB],
                    )
                nc.sync.dma_start(out=out[tt], in_=ot)

    nc.compile()
    return nc


def make_in_maps(hidden_states, Wqkv, bqkv, Wproj, bproj, splat_scale, splat_bias):
    hs = np.asarray(hidden_states, dtype=np.float32)
    Wqkv = np.asarray(Wqkv, dtype=np.float32)
    bqkv = np.asarray(bqkv, dtype=np.float32)
    Wproj = np.asarray(Wproj, dtype=np.float32)
    bproj = np.asarray(bproj, dtype=np.float32)
    s = (1.0 + 0.01 * np.tanh(np.asarray(splat_scale, dtype=np.float32))).astype(np.float32)
    bsp = (0.001 * np.tanh(np.asarray(splat_bias, dtype=np.float32).reshape(H))).astype(np.float32)
    scale_factor = np.float32(1.0 / math.sqrt(D))

    Wq, Wk, Wv = Wqkv[0:E], Wqkv[E:2 * E], Wqkv[2 * E:3 * E]
    bq, bk, bv = bqkv[0:E], bqkv[E:2 * E], bqkv[2 * E:3 * E]

    tri = np.where(np.arange(P)[None, :] >= np.arange(P)[:, None],
                   np.float32(0.0), np.float32(-1e30)).astype(np.float32)

    group_maps = []
    for g in range(2):
        gs = slice(g * F, (g + 1) * F)
        wqkT = np.ascontiguousarray(
            np.concatenate([Wq[gs], Wk[gs]], axis=0).T).astype(np.float32)
        wvT = np.ascontiguousarray(Wv[gs].T).astype(np.float32)
        qk_bias = np.ascontiguousarray(
            np.concatenate([bq[gs], bk[gs]]).reshape(FT_QK, P).T).astype(np.float32)
        v_bias = np.ascontiguousarray(bv[gs].reshape(1, F)).astype(np.float32)
        wpT = np.ascontiguousarray(Wproj[:, gs].T).astype(np.float32)
        bp = (bproj * 0.5).reshape(1, E).astype(np.float32)
        hsl = slice(g * HG, (g + 1) * HG)
        act_s = np.tile((s[hsl] * scale_factor).reshape(1, HG), (P, 1)).astype(np.float32)
        act_b = np.tile(bsp[hsl].reshape(1, HG), (P, 1)).astype(np.float32)
        group_maps.append(dict(
            wqkT=wqkT, wvT=wvT, qk_bias=qk_bias, v_bias=v_bias,
            wpT=wpT, bp_half=bp, tri=tri, act_s=act_s, act_b=act_b,
        ))

    in_maps = []
    for c in range(8):
        b, g = c // 2, c % 2
        m = dict(group_maps[g])
        m["hsT"] = np.ascontiguousarray(hs[b].T).astype(np.float32)
        in_maps.append(m)
    return in_maps


def kernel(hidden_states, Wqkv, bqkv, Wproj, bproj, splat_scale, splat_bias,
           **run_kwargs):
    in_maps = make_in_maps(hidden_states, Wqkv, bqkv, Wproj, bproj,
                           splat_scale, splat_bias)
    nc = build_program()
    res = run_bass_kernel_spmd(nc, in_maps, core_ids=list(range(8)), **run_kwargs)
    outs = [np.asarray(r["out"], dtype=np.float32).reshape(T, E) for r in res.results]
    full = np.stack([outs[2 * b] + outs[2 * b + 1] for b in range(B)], axis=0)
    return full



# revision 9
# speedup vs baseline: 1.6119x; 1.6119x over previous
"""Trainium2 Bass kernel for ConservativeGSAAttention (v2).

Sharding: 8 cores = 4 batches x 2 head-groups (8 heads each).
Each core computes qkv-proj + attention + its half of c_proj for one batch;
the host sums the two partial c_proj outputs per batch (the "all-reduce").

v2 vs v1 (763us baseline):
  - Fully SBUF-resident: q/k/v/at/ao never round-trip through DRAM, so the
    PE is never starved and the HAM clock gate stays at K=8/8 (2.4 GHz).
    v1 spent ~500us at K=4/8 because DMA waits between phases re-throttled
    the PE every phase.
  - bf16 operands everywhere (inputs converted on host): halves DMA + SBUF,
    enables FWL weight loads. Matmul accumulation stays fp32 in PSUM.
  - Score matmuls have K=D=64 contraction; two heads are packed into the
    128-row PE array concurrently (row tiling via base_partition 0/64), so
    a head-pair costs one matmul slot instead of two.
  - softmax exp runs on ACT over 2-tile spans straight out of PSUM; the
    causal triangle is applied by DVE adding -1e30 tri masks in-place in
    PSUM before the exp.  Above-diagonal zeros in `at` come from
    persistent zero-initialized tiles that are never rewritten.
  - softmax denominator = row 64 of the AV matmul (ones column in v).
    Normalization: reciprocal of that row (DVE) -> outer-product matmul
    broadcasts it into rows 64:128 of the same PSUM bank -> DVE multiply.
  - Phases are interleaved: qkv-projection of token block tb+1 and
    c_proj of block qb are emitted inside the attention loop over qb, so
    the PE always has independent work while ACT chews on exps.
"""

import math
from contextlib import ExitStack

import numpy as np

import concourse.bass as bass
import concourse.tile as tile
from concourse import bacc
from concourse import mybir
from concourse.bass_utils import run_bass_kernel_spmd

B, T, E, H, D = 4, 2048, 1024, 16, 64
HG = 8              # heads per core
F = HG * D          # 512 feats per group (for each of q, k, v)
P = 128
KT = E // P         # 8 contraction tiles for projections
QB = 512            # query block width
NQB = T // QB       # 4
TT = T // P         # 16 token tiles
NFP = 4             # feat-pair tiles (128 feats = 2 heads each)

f32 = mybir.dt.float32
f32r = mybir.dt.float32r
bf16 = mybir.dt.bfloat16

AF = mybir.ActivationFunctionType


def build_program():
    nc = bacc.Bacc("TRN2", target_bir_lowering=False, debug=False)

    # ---- I/O ----
    hsT = nc.dram_tensor("hsT", [E, T], bf16, kind="ExternalInput").ap()
    wqkT = nc.dram_tensor("wqkT", [E, 2 * F], bf16, kind="ExternalInput").ap()
    wvT = nc.dram_tensor("wvT", [E, F], bf16, kind="ExternalInput").ap()
    wpT = nc.dram_tensor("wpT", [F, E], bf16, kind="ExternalInput").ap()
    qkb = nc.dram_tensor("qkb", [P, 2 * NFP], f32, kind="ExternalInput").ap()
    vb = nc.dram_tensor("vb", [1, F], bf16, kind="ExternalInput").ap()
    bp = nc.dram_tensor("bp", [1, E], f32, kind="ExternalInput").ap()
    tri = nc.dram_tensor("tri", [P, P], f32, kind="ExternalInput").ap()
    act_s = nc.dram_tensor("act_s", [P, HG], f32, kind="ExternalInput").ap()
    act_b = nc.dram_tensor("act_b", [P, HG], f32, kind="ExternalInput").ap()
    out = nc.dram_tensor("out", [TT, P, E], f32, kind="ExternalOutput").ap()

    with tile.TileContext(nc) as tc, ExitStack() as ctx:
        def pool(name, bufs, space="SBUF"):
            return ctx.enter_context(tc.tile_pool(name=name, bufs=bufs, space=space))

        consts = pool("consts", 1)
        persist = pool("persist", 1)
        hs_pool = pool("hs", 2)
        at_pool = pool("at", 4)
        rc_pool = pool("rc", 4)
        bc_pool = pool("bc", 4)
        ot_pool = pool("ot", 2)
        ps_big = pool("ps_big", 3, "PSUM")
        ps_av = pool("ps_av", 2, "PSUM")

        # ---- resident constants (split DMAs so A(0) can start early) ----
        wqk_sb = consts.tile([P, KT, 2 * F], bf16, tag="wqk")
        wqkT_t = wqkT.rearrange("(kt p) f -> p kt f", p=P)
        for kt in range(KT):
            nc.sync.dma_start(out=wqk_sb[:, kt, :], in_=wqkT_t[:, kt, :])
        wv_sb = consts.tile([P, KT, F], bf16, tag="wv")
        nc.sync.dma_start(out=wv_sb, in_=wvT.rearrange("(kt p) f -> p kt f", p=P))
        qkb_sb = consts.tile([P, 2 * NFP], f32, tag="qkb")
        nc.sync.dma_start(out=qkb_sb, in_=qkb)
        vb_sb = consts.tile([P, 1, F], bf16, tag="vb")
        nc.sync.dma_start(out=vb_sb, in_=vb.partition_broadcast(P))
        tri_sb = consts.tile([P, P], f32, tag="tri")
        nc.sync.dma_start(out=tri_sb, in_=tri)
        acts_sb = consts.tile([P, HG], f32, tag="acts")
        nc.sync.dma_start(out=acts_sb, in_=act_s)
        actb_sb = consts.tile([P, HG], f32, tag="actb")
        nc.sync.dma_start(out=actb_sb, in_=act_b)
        wp_sb = consts.tile([P, NFP, E], bf16, tag="wp")
        nc.sync.dma_start(out=wp_sb, in_=wpT.rearrange("(ft p) e -> p ft e", p=P))
        bp_sb = consts.tile([P, 1, E], f32, tag="bp")
        nc.sync.dma_start(out=bp_sb, in_=bp.partition_broadcast(P))


        # ---- persistent SBUF state ----
        # q/k in transposed layout [feat, token]; partition = feat within a
        # head-pair tile; one tile per token block so deps stay narrow.
        qT = [persist.tile([P, NFP, QB], bf16, tag=f"qT{tb}", name=f"qT{tb}") for tb in range(NQB)]
        kT = [persist.tile([P, NFP, QB], bf16, tag=f"kT{tb}", name=f"kT{tb}") for tb in range(NQB)]
        # v per token tile: [token, head, D+1]; col D is the ones column for
        # the softmax denominator, initialized once.
        v_sb = [persist.tile([P, HG, D + 1], bf16, tag=f"v{tt}", name=f"v{tt}") for tt in range(TT)]
        for tt in range(TT):
            nc.vector.memset(v_sb[tt][:, :, D:D + 1], 1.0)
        # persistent diagonal `at` tiles: cols < j*128 stay zero forever.
        # 2 sets x 2 heads-in-pair x 4 diagonal positions.
        atd = [[[persist.tile([P, QB], bf16, tag=f"atd{s}_{c}_{j}", name=f"atd{s}_{c}_{j}")
                 for j in range(4)] for c in range(2)] for s in range(2)]
        for s in range(2):
            for c in range(2):
                for j in range(4):
                    nc.vector.memset(atd[s][c][j], 0.0)
        # attention output [feat, token] per query block, double-buffered.
        ao = [persist.tile([P, NFP, QB], bf16, tag=f"ao{i}", name=f"ao{i}") for i in range(2)]

        hsT_t = hsT.rearrange("(kt p) t -> p kt t", p=P)

        # ---- phase A unit generators (qkv projection for token block tb) ----
        def a_load(tb):
            hs_t = hs_pool.tile([P, KT, QB], bf16, tag="hs")
            half = KT // 2
            nc.sync.dma_start(
                out=hs_t[:, 0:half, :],
                in_=hsT_t[:, 0:half, tb * QB:(tb + 1) * QB])
            nc.sync.dma_start(
                out=hs_t[:, half:KT, :],
                in_=hsT_t[:, half:KT, tb * QB:(tb + 1) * QB])
            return hs_t

        def a_qk_unit(tb, hs_t, g, is_k):
            # projects feat-pair tiles 2g, 2g+1 of q (or k) for block tb
            col0 = F if is_k else 0
            dst = kT[tb] if is_k else qT[tb]
            b0 = 4 * is_k + 2 * g
            ps = ps_big.tile([P, 2, QB], f32, tag="big")
            for fi in range(2):
                fp = 2 * g + fi
                for kt in range(KT):
                    nc.tensor.matmul(
                        ps[:, fi, :],
                        wqk_sb[:, kt, col0 + fp * P:col0 + (fp + 1) * P],
                        hs_t[:, kt, :],
                        start=(kt == 0), stop=(kt == KT - 1),
                    )
            for fi in range(2):
                fp = 2 * g + fi
                nc.vector.tensor_scalar_add(
                    dst[:, fp, :], ps[:, fi, :], qkb_sb[:, b0 + fi:b0 + fi + 1])

        def a_v_unit(tb, hs_t, u):
            # projects v for token tiles tb*4 + 2u, +2u+1
            ps = ps_big.tile([P, 2, QB], f32, tag="big")
            for ti in range(2):
                tsub = 2 * u + ti
                for kt in range(KT):
                    nc.tensor.matmul(
                        ps[:, ti, :],
                        hs_t[:, kt, tsub * P:(tsub + 1) * P],
                        wv_sb[:, kt, :],
                        start=(kt == 0), stop=(kt == KT - 1),
                    )
            for ti in range(2):
                tt = tb * 4 + 2 * u + ti
                nc.vector.tensor_add(
                    v_sb[tt][:, :, 0:D],
                    ps[:, ti, :].rearrange("p (h d) -> p h d", h=HG),
                    vb_sb[:, 0, :].rearrange("p (h d) -> p h d", h=HG),
                )

        def a_units(tb):
            hs_t = a_load(tb)
            units = []
            for g in range(2):
                units.append(lambda g=g: a_qk_unit(tb, hs_t, g, False))
            for g in range(2):
                units.append(lambda g=g: a_qk_unit(tb, hs_t, g, True))
            for u in range(2):
                units.append(lambda u=u: a_v_unit(tb, hs_t, u))
            return units

        # ---- phase B: attention for head pair hp, query block qb ----
        def b_headpair(qb, hp):
            nkt = (qb + 1) * 4
            seti = (qb * 4 + hp) % 2
            av = [ps_av.tile([P, QB], f32, tag="av", name="av") for _ in range(2)]
            nspan = nkt // 2
            sc_ps = [None] * nspan
            sc_at = [None] * nspan

            def emit_scores(s):
                kt0 = 2 * s
                ps2 = [ps_big.tile([P, 2, QB], f32, tag="big", name="scps") for _ in range(2)]
                for j2 in range(2):
                    kt = kt0 + j2
                    tbk, ktk = kt // 4, kt % 4
                    for c in range(2):
                        nc.tensor.matmul(
                            ps2[c][:, j2, :],
                            kT[tbk][64 * c:64 * (c + 1), hp, ktk * P:(ktk + 1) * P],
                            qT[qb][64 * c:64 * (c + 1), hp, :],
                            start=True, stop=True,
                        )
                sc_ps[s] = ps2

            def emit_exp(s):
                kt0 = 2 * s
                ps2 = sc_ps[s]
                diag = kt0 >= qb * 4
                ats = []
                if not diag:
                    for c in range(2):
                        h = 2 * hp + c
                        at = at_pool.tile([P, 2, QB], bf16, tag="at")
                        nc.scalar.activation(
                            out=at, in_=ps2[c], func=AF.Exp,
                            bias=actb_sb[:, h:h + 1], scale=acts_sb[:, h:h + 1])
                        ats.append((at[:, 0, :], at[:, 1, :]))
                else:
                    for c in range(2):
                        h = 2 * hp + c
                        pair = []
                        for j2 in range(2):
                            kt = kt0 + j2
                            j = kt - qb * 4
                            nc.vector.tensor_add(
                                ps2[c][:, j2, j * P:(j + 1) * P],
                                ps2[c][:, j2, j * P:(j + 1) * P],
                                tri_sb)
                            dst = atd[seti][c][j]
                            nc.scalar.activation(
                                out=dst[:, j * P:QB],
                                in_=ps2[c][:, j2, j * P:QB], func=AF.Exp,
                                bias=actb_sb[:, h:h + 1], scale=acts_sb[:, h:h + 1])
                            pair.append(dst)
                        ats.append(tuple(pair))
                sc_at[s] = ats

            def emit_av(s):
                kt0 = 2 * s
                for j2 in range(2):
                    kt = kt0 + j2
                    for c in range(2):
                        h = 2 * hp + c
                        nc.tensor.matmul(
                            av[c][0:D + 1, :],
                            v_sb[kt][:, h, :],
                            sc_at[s][c][j2],
                            start=(kt == 0), stop=(kt == nkt - 1),
                        )

            # software pipeline: PE does scores(s+1) while ACT exps span s,
            # then av(s) lands with its `at` already in SBUF.
            emit_scores(0)
            emit_exp(0)
            for s in range(1, nspan):
                emit_scores(s)
                emit_av(s - 1)
                emit_exp(s)
            emit_av(nspan - 1)

            # normalization: rows 0..63 of av divided by row 64 (exp-sum).
            # The reciprocal row is broadcast across partitions on the
            # otherwise-idle GPSIMD engine (DVE cannot read 2 PSUM inputs).
            for c in range(2):
                rc = rc_pool.tile([1, QB], f32, tag="rc")
                nc.vector.reciprocal(rc, av[c][D:D + 1, :])
                bc = bc_pool.tile([D, QB], f32, tag="bc")
                nc.gpsimd.partition_broadcast(bc, rc, channels=D)
                nc.vector.tensor_mul(
                    ao[qb % 2][64 * c:64 * (c + 1), hp, :],
                    av[c][0:D, :], bc)

        # ---- phase C: c_proj for the 4 token tiles of query block qb ----
        def c_block(qb):
            for ts in range(4):
                tt = qb * 4 + ts
                psc = ps_big.tile([P, 2, QB], f32, tag="big")
                for eb in range(2):
                    for fp in range(NFP):
                        nc.tensor.matmul(
                            psc[:, eb, :],
                            ao[qb % 2][:, fp, ts * P:(ts + 1) * P],
                            wp_sb[:, fp, eb * QB:(eb + 1) * QB],
                            start=(fp == 0), stop=(fp == NFP - 1),
                        )
                ot = ot_pool.tile([P, E], f32, tag="ot")
                nc.vector.tensor_add(
                    ot, psc.rearrange("p a b -> p (a b)"), bp_sb[:, 0, :])
                nc.sync.dma_start(out=out[tt], in_=ot)

        # ---- main schedule ----
        for unit in a_units(0):
            unit()
        pending = []
        for qb in range(NQB):
            if qb < NQB - 1:
                pending = a_units(qb + 1)
            for hp in range(4):
                # feed ~2 qkv-projection units of the NEXT token block per
                # head pair so the PE always has independent work queued.
                take = 2 if hp < 2 else 1
                for _ in range(take):
                    if pending:
                        pending.pop(0)()
                b_headpair(qb, hp)
            while pending:
                pending.pop(0)()
            c_block(qb)

    nc.compile()
    return nc


def make_in_maps(hidden_states, Wqkv, bqkv, Wproj, bproj, splat_scale, splat_bias):
    np_bf16 = mybir.dt.np(bf16)
    hs = np.asarray(hidden_states, dtype=np.float32)
    Wqkv = np.asarray(Wqkv, dtype=np.float32)
    bqkv = np.asarray(bqkv, dtype=np.float32)
    Wproj = np.asarray(Wproj, dtype=np.float32)
    bproj = np.asarray(bproj, dtype=np.float32)
    s = (1.0 + 0.01 * np.tanh(np.asarray(splat_scale, dtype=np.float32))).astype(np.float32)
    bsp = (0.001 * np.tanh(np.asarray(splat_bias, dtype=np.float32).reshape(H))).astype(np.float32)
    scale_factor = np.float32(1.0 / math.sqrt(D))

    Wq, Wk, Wv = Wqkv[0:E], Wqkv[E:2 * E], Wqkv[2 * E:3 * E]
    bq, bk, bv = bqkv[0:E], bqkv[E:2 * E], bqkv[2 * E:3 * E]

    tri = np.where(np.arange(P)[None, :] >= np.arange(P)[:, None],
                   np.float32(0.0), np.float32(-1e30)).astype(np.float32)

    group_maps = []
    for g in range(2):
        gs = slice(g * F, (g + 1) * F)
        wqkT = np.ascontiguousarray(
            np.concatenate([Wq[gs], Wk[gs]], axis=0).T).astype(np_bf16)
        wvT = np.ascontiguousarray(Wv[gs].T).astype(np_bf16)
        qk_bias = np.ascontiguousarray(
            np.concatenate([bq[gs], bk[gs]]).reshape(2 * NFP, P).T).astype(np.float32)
        v_bias = np.ascontiguousarray(bv[gs].reshape(1, F)).astype(np_bf16)
        wpT = np.ascontiguousarray(Wproj[:, gs].T).astype(np_bf16)
        bp_half = (bproj * 0.5).reshape(1, E).astype(np.float32)
        hsl = slice(g * HG, (g + 1) * HG)
        act_s = np.tile((s[hsl] * scale_factor).reshape(1, HG), (P, 1)).astype(np.float32)
        act_b = np.tile(bsp[hsl].reshape(1, HG), (P, 1)).astype(np.float32)
        group_maps.append(dict(
            wqkT=wqkT, wvT=wvT, qkb=qk_bias, vb=v_bias,
            wpT=wpT, bp=bp_half, tri=tri, act_s=act_s, act_b=act_b,
        ))

    in_maps = []
    for c in range(8):
        b, g = c // 2, c % 2
        m = dict(group_maps[g])
        m["hsT"] = np.ascontiguousarray(hs[b].T).astype(np_bf16)
        in_maps.append(m)
    return in_maps


def kernel(hidden_states, Wqkv, bqkv, Wproj, bproj, splat_scale, splat_bias,
           **run_kwargs):
    in_maps = make_in_maps(hidden_states, Wqkv, bqkv, Wproj, bproj,
                           splat_scale, splat_bias)
    nc = build_program()
    res = run_bass_kernel_spmd(nc, in_maps, core_ids=list(range(8)), **run_kwargs)
    outs = [np.asarray(r["out"], dtype=np.float32).reshape(T, E) for r in res.results]
    full = np.stack([outs[2 * b] + outs[2 * b + 1] for b in range(B)], axis=0)
    return full
